# revision 54
# baseline (speedup 1.0000x reference)
"""GAT (graph attention) layer on 8 TRN2 NeuronCores.

Algorithm (mathematically equal to the reference):
  proj = in_feat @ W_proj;  s = proj @ A_src;  t = proj @ A_tgt
  per edge e=(src,tgt): att_e = exp(leakyrelu(s[src] + t[tgt]) - SHIFT)
                              = max(exp(x-SHIFT), exp(0.2x-SHIFT)), x = s+t
  out[tgt] = (sum_e att_e * proj[src]) / (sum_e att_e + eps) + bias

The reference subtracts the global max of the pre-activation scores before
exp(); since numerator and denominator scale identically, any constant shift
yields the same output.  Input scales are fixed by the problem spec, so
scores lie in ~[-11, 11]; SHIFT=16 keeps exp() in fp32 range.

Sharding: edges are sharded by TARGET node so each core owns a disjoint
output slice; no collective needed.  Nodes are relabeled into "row space"
r(n) = (n%128)*391 + n//128 so phase-1 (projection) streams its tables to
HBM with large contiguous descriptors per partition.  dma_gather indices
are int16 so tables split at row 32768 (lo/hi); each core owns 32 blocks
of 128 lo targets and 17 blocks of 128 hi targets.

Phase 2, per 128-target block:
  - gather per-edge proj rows (256B) and per-edge src-score rows (256B)
  - gather the block's 128 target score rows once (tiny), and broadcast
    t[tgt] to edges with per-tile one-hot matmuls (host-provided fp8
    transposed one-hots), accumulating into PSUM
  - att on ACT (two exps) + DVE max; weighted features on DVE (bf16 2x)
  - segment-sum into PSUM with one-hot matmuls (host-provided fp8 one-hots,
    one matmul per 128-edge tile)
Epilogue is batched: all 49 block accumulators stage in SBUF, then one
reciprocal/scale/bias pass and a single output write.
"""
import sys
sys.path.insert(0, "/opt/trn_rl_repo")
import numpy as np

import concourse.bass as bass
import concourse.bacc as bacc
import concourse.mybir as mybir
import concourse.tile as tile
from concourse._compat import cdiv
from concourse.library_config import mlp

P = 128
N_NODES = 50000
N_CORES = 8
NT_NODE = 391                  # node tiles; 128*391 = 50048
NPAD = P * NT_NODE             # 50048
SPLIT = 32768                  # int16-safe table split (row space)
LO_NBLK = 32                   # lo blocks per core (32*128*8 = 32768 rows)
HI_NBLK = 17
NBLK = LO_NBLK + HI_NBLK       # 49
LO_TPC = SPLIT // N_CORES      # 4096 lo rows per core
HI_TPC = (NPAD - SPLIT) // N_CORES  # 2160 hi rows per core
D = 128
H = 4
SHIFT = 16.0
EPS = 1e-16

_cache = {}

CFG = {
    "pk": 4,
    "slab": 12,
    "stg_bufs": 4,
    "g_bufs": 3,
    "wk_bufs": 3,
    "s_bufs": 3,
}


def _r_of(n):
    """node id -> table row (r-space)."""
    return (n % P) * NT_NODE + n // P


def _build(k_lo, k_hi):
    nc = bacc.Bacc("TRN2", target_bir_lowering=False, debug=False)
    f32, bf16 = mybir.dt.float32, mybir.dt.bfloat16
    i16, fp8 = mybir.dt.int16, mybir.dt.float8e4

    T_B = k_lo + k_hi
    IW = T_B * 8                 # int16 idx cols per block (wrapped /16)
    SW = T_B * P                 # one-hot cols per block

    xT_d = nc.dram_tensor("xT", [P, NPAD], bf16, kind="ExternalInput")
    W_d = nc.dram_tensor("W", [P, D], bf16, kind="ExternalInput")
    WT_d = nc.dram_tensor("WT", [P, D], bf16, kind="ExternalInput")
    A_d = nc.dram_tensor("A", [P, 2 * H], bf16, kind="ExternalInput")
    bias_d = nc.dram_tensor("bias", [1, D], f32, kind="ExternalInput")
    sidx_d = nc.dram_tensor("sidx", [P, NBLK * IW], i16, kind="ExternalInput")
    qidx_d = nc.dram_tensor("qidx", [P, NBLK * 8], i16, kind="ExternalInput")
    S8_d = nc.dram_tensor("S8", [P, NBLK * SW], fp8, kind="ExternalInput")
    St8_d = nc.dram_tensor("St8", [P, NBLK * SW], fp8, kind="ExternalInput")
    out_d = nc.dram_tensor("out", [NBLK * P, D], f32, kind="ExternalOutput")

    # proj table: row r = 64 f32 = 128 bf16 (head-interleaved proj), 256B
    t1 = nc.dram_tensor("t1", [NPAD, 64], f32)
    # score table: row r = 64 f32, cells 0:8 used = [s[4] | t[4]] f32
    mt = nc.dram_tensor("mt", [NPAD, 64], f32)

    PK = CFG["pk"]
    SLAB = CFG["slab"]

    with tile.TileContext(nc) as tc:
        with (
            tc.tile_pool(name="const", bufs=1) as cp,
            tc.tile_pool(name="p1x", bufs=CFG.get("p1x_bufs", 4)) as p1x,
            tc.tile_pool(name="stg", bufs=CFG["stg_bufs"]) as stg,
            tc.tile_pool(name="p1ps", bufs=4, space="PSUM") as p1ps,
            tc.tile_pool(name="acc", bufs=2, space="PSUM") as accp,
            tc.tile_pool(name="taups", bufs=2, space="PSUM") as taups,
            tc.tile_pool(name="g", bufs=CFG["g_bufs"]) as g,
            tc.tile_pool(name="sp", bufs=CFG["s_bufs"]) as spool,
            tc.tile_pool(name="wk", bufs=CFG["wk_bufs"]) as wk,
        ):
            nc.gpsimd.load_library(mlp)
            # ---- constants ----
            nshift = cp.tile([P, 1], f32)
            nc.gpsimd.memset(nshift[:], -SHIFT)
            sidx = cp.tile([P, NBLK * IW], i16)
            qidx = cp.tile([P, NBLK * 8], i16)
            nc.sync.dma_start(sidx[:], sidx_d[:])
            nc.sync.dma_start(qidx[:], qidx_d[:])

            # W_ext = [W | W@A] (136 cols, bf16)
            W_ext = cp.tile([P, D + 2 * H], bf16)
            nc.sync.dma_start(W_ext[:, :D], W_d[:])
            WT_sb = cp.tile([P, D], bf16)
            A_sb = cp.tile([P, 2 * H], bf16)
            nc.sync.dma_start(WT_sb[:], WT_d[:])
            nc.sync.dma_start(A_sb[:], A_d[:])
            wa_ps = accp.tile([P, 2 * H], f32, tag="acc")
            nc.tensor.matmul(out=wa_ps[:], lhsT=WT_sb[:], rhs=A_sb[:],
                             start=True, stop=True)
            nc.vector.tensor_copy(out=W_ext[:, D:], in_=wa_ps[:])

            # bias broadcast to all partitions (host pre-interleaves cols)
            ones_row = cp.tile([1, P], f32)
            nc.gpsimd.memset(ones_row[:], 1.0)
            bias_row = cp.tile([1, D], f32)
            nc.sync.dma_start(bias_row[:], bias_d[:])
            bias_ps = accp.tile([P, D], f32, tag="acc")
            nc.tensor.matmul(out=bias_ps[:], lhsT=ones_row[:], rhs=bias_row[:],
                             start=True, stop=True)
            bias_mat = cp.tile([P, D], f32)
            nc.vector.tensor_copy(out=bias_mat[:], in_=bias_ps[:])

            # output staging + phase-1 score staging (written once to mt)
            ostage = cp.tile([P, NBLK * (D + H)], f32)
            rec = cp.tile([P, NBLK * H], f32)
            mstage = cp.tile([P, NT_NODE * 8], f32)

            # ---- phase 1: projection + scores -> packed tables ----
            t1v = t1[:].rearrange("(p j) c -> p j c", p=P)
            mtv = mt[:].rearrange("(p j) c -> p j c", p=P)
            slabs = {}
            for s in range(0, NT_NODE, SLAB):
                w = min(SLAB, NT_NODE - s)
                xs = p1x.tile([P, SLAB * P], bf16, tag="xslab")
                nc.sync.dma_start(xs[:, :w * P], xT_d[:, s * P:(s + w) * P])
                slabs[s] = xs
            gi = 0
            # 3 node-tiles share one PSUM bank (136-col stride) so 4 p1
            # buffers fit; t1 writes batch 6 tiles (>=512B descriptors)
            for wbase in range(0, NT_NODE, 6):
                wn = min(6, NT_NODE - wbase)
                pstg = stg.tile([P, 6 * 64], f32, tag="pstg")
                for sub in range(0, wn, 3):
                    k = min(3, wn - sub)
                    base = wbase + sub
                    ps = p1ps.tile([P, 512], f32, tag="p1")
                    for j in range(k):
                        nt = base + j
                        xs = slabs[(nt // SLAB) * SLAB]
                        o = (nt % SLAB) * P
                        nc.tensor.matmul(
                            out=ps[:, j * 136:(j + 1) * 136],
                            lhsT=xs[:, o:o + P], rhs=W_ext[:],
                            start=True, stop=True)
                    ps_r = ps[:, :408].rearrange("p (j c) -> p j c", j=3)[:, :k]
                    out_v = pstg[:, sub * 64:(sub + k) * 64].bitcast(bf16)                        .rearrange("p (j r h) -> p j r h", j=k, h=H)
                    in_v = ps_r[:, :, 0:D].rearrange("p j (h r) -> p j r h", h=H)
                    # proj -> bf16 head-interleaved + scores, alternating engines
                    if gi % 2 == 0:
                        nc.vector.tensor_copy(out=out_v, in_=in_v)
                        nc.vector.tensor_copy(
                            out=mstage[:, base * 8:(base + k) * 8],
                            in_=ps_r[:, :, D:D + 2 * H])
                    else:
                        nc.scalar.activation(
                            out=out_v, in_=in_v,
                            func=mybir.ActivationFunctionType.Copy)
                        nc.scalar.activation(
                            out=mstage[:, base * 8:(base + k) * 8],
                            in_=ps_r[:, :, D:D + 2 * H],
                            func=mybir.ActivationFunctionType.Copy)
                    gi += 1
                nc.gpsimd.dma_start(
                    t1v[:, wbase:wbase + wn, :],
                    pstg[:].rearrange("p (j c) -> p j c", j=6)[:, :wn])
                # chunked mt flush (unblocks phase-2 sooner than one big write)
                if (wbase + wn) % 66 == 0 or wbase + wn == NT_NODE:
                    lo = ((wbase + wn - 1) // 66) * 66
                    nc.gpsimd.dma_start(
                        mtv[:, lo:wbase + wn, 0:8],
                        mstage[:, lo * 8:(wbase + wn) * 8]
                            .rearrange("p (j c) -> p j c", c=8))

            # ---- phase 2: per 128-target block ----
            t1lo, t1hi = t1[:SPLIT], t1[SPLIT:]
            mlo, mhi = mt[:SPLIT], mt[SPLIT:]
            for b in range(NBLK):
                is_lo = b < LO_NBLK
                gi0 = b * IW
                prows = g.tile([P, T_B * 64], f32, tag="prows")
                msrc = g.tile([P, T_B * 64], f32, tag="msrc")
                tsc = g.tile([P, 64], f32, tag="tsc")
                # block target score rows first (tau chain depends on it)
                nc.gpsimd.dma_gather(
                    tsc[:].rearrange("p (k c) -> p k c", k=1),
                    mlo if is_lo else mhi, qidx[:, b * 8:(b + 1) * 8],
                    P, P, 64, single_packet=False)
                if k_lo:
                    nc.gpsimd.dma_gather(
                        prows[:, :k_lo * 64].rearrange("p (k c) -> p k c", k=k_lo),
                        t1lo, sidx[:, gi0:gi0 + k_lo * 8],
                        k_lo * P, k_lo * P, 64, single_packet=False)
                    nc.gpsimd.dma_gather(
                        msrc[:, :k_lo * 64].rearrange("p (k c) -> p k c", k=k_lo),
                        mlo, sidx[:, gi0:gi0 + k_lo * 8],
                        k_lo * P, k_lo * P, 64, single_packet=False)
                if k_hi:
                    nc.gpsimd.dma_gather(
                        prows[:, k_lo * 64:].rearrange("p (k c) -> p k c", k=k_hi),
                        t1hi, sidx[:, gi0 + k_lo * 8:gi0 + IW],
                        k_hi * P, k_hi * P, 64, single_packet=False)
                    nc.gpsimd.dma_gather(
                        msrc[:, k_lo * 64:].rearrange("p (k c) -> p k c", k=k_hi),
                        mhi, sidx[:, gi0 + k_lo * 8:gi0 + IW],
                        k_hi * P, k_hi * P, 64, single_packet=False)
                # one-hots for this block
                S_t = spool.tile([P, SW], fp8, tag="S")
                St_t = spool.tile([P, SW], fp8, tag="St")
                nc.sync.dma_start(S_t[:], S8_d[:, b * SW:(b + 1) * SW])
                nc.scalar.dma_start(St_t[:], St8_d[:, b * SW:(b + 1) * SW])

                # broadcast t[tgt] to edges: per tile j one-hot matmul
                tsc_b = wk.tile([P, H], bf16, tag="tscb")
                nc.scalar.activation(out=tsc_b[:], in_=tsc[:, H:2 * H],
                                     func=mybir.ActivationFunctionType.Copy)
                tau = taups.tile([P, T_B * H], f32, tag="tau")
                for j in range(T_B):
                    nc.tensor.matmul(
                        out=tau[:, j * H:(j + 1) * H],
                        lhsT=St_t[:, j * P:(j + 1) * P],
                        rhs=tsc_b[:],
                        start=True, stop=True)

                msrc_f = msrc[:].rearrange("p (j c) -> p j c", j=T_B)
                # x = s_src + t_tgt ; att = max(exp(x-S), exp(0.2x-S))
                xy = wk.tile([P, 2 * T_B * H], f32, tag="xy")
                ey = wk.tile([P, 2 * T_B * H], bf16, tag="ey")
                wide = wk.tile([P, T_B * (D + H)], bf16, tag="wide")
                wide_r = wide[:].rearrange("p (j c) -> p j c", j=T_B)
                nc.vector.tensor_tensor(
                    out=xy[:, :T_B * H].rearrange("p (j h) -> p j h", j=T_B),
                    in0=msrc_f[:, :, 0:H],
                    in1=tau[:].rearrange("p (j h) -> p j h", j=T_B),
                    op=mybir.AluOpType.add)
                nc.vector.tensor_scalar(
                    out=xy[:, T_B * H:], in0=xy[:, :T_B * H], scalar1=0.2,
                    scalar2=None, op0=mybir.AluOpType.mult)
                nc.scalar.activation(out=ey[:],
                                     in_=xy[:],
                                     func=mybir.ActivationFunctionType.Exp,
                                     bias=nshift[:])
                nc.vector.tensor_tensor(
                    out=wide_r[:, :, D:],
                    in0=ey[:, :T_B * H].rearrange("p (j h) -> p j h", j=T_B),
                    in1=ey[:, T_B * H:].rearrange("p (j h) -> p j h", j=T_B),
                    op=mybir.AluOpType.max)
                # weighted features: proj (head-interleaved) * att
                # (two halves so the scatter matmuls start sooner)
                TH = T_B // 2
                for h0, h1 in ((0, TH), (TH, T_B)):
                    nc.vector.tensor_tensor(
                        out=wide_r[:, h0:h1, 0:D].rearrange(
                            "p j (r h) -> p j r h", h=H),
                        in0=prows[:].bitcast(bf16)
                            .rearrange("p (j r h) -> p j r h", j=T_B, h=H)[:, h0:h1],
                        in1=wide_r[:, h0:h1, D:].unsqueeze(2)
                            .to_broadcast([P, h1 - h0, D // H, H]),
                        op=mybir.AluOpType.mult)
                acc = accp.tile([P, D + H], f32, tag="acc")
                for j in range(T_B):
                    nc.tensor.matmul(
                        out=acc[:],
                        lhsT=S_t[:, j * P:(j + 1) * P],
                        rhs=wide[:, j * (D + H):(j + 1) * (D + H)],
                        start=(j == 0), stop=(j == T_B - 1))
                # stage accumulator (EPS keeps pad-slot denominators nonzero)
                nc.scalar.activation(
                    out=ostage[:, b * (D + H):(b + 1) * (D + H)],
                    in_=acc[:], func=mybir.ActivationFunctionType.Copy,
                    bias=float(EPS))
                if b % 8 == 7 or b == NBLK - 1:
                    # epilogue for the finished chunk: out = num/den + bias
                    b0, b1 = (b // 8) * 8, min(b + 1, NBLK)
                    if b == NBLK - 1:
                        b0 = (NBLK // 8) * 8
                    nb = b1 - b0
                    ost_r = ostage[:].rearrange("p (b c) -> p b c", b=NBLK)
                    rec_r = rec[:].rearrange("p (b h) -> p b h", b=NBLK)
                    nc.vector.reciprocal(rec_r[:, b0:b1], ost_r[:, b0:b1, D:])
                    eeng = nc.vector if b < NBLK - 1 else nc.gpsimd
                    eeng.tensor_tensor(
                        out=ost_r[:, b0:b1, 0:D].rearrange(
                            "p b (r h) -> p b r h", h=H),
                        in0=ost_r[:, b0:b1, 0:D].rearrange(
                            "p b (r h) -> p b r h", h=H),
                        in1=rec_r[:, b0:b1].unsqueeze(2)
                            .to_broadcast([P, nb, D // H, H]),
                        op=mybir.AluOpType.mult)
                    eeng.tensor_tensor(
                        out=ost_r[:, b0:b1, 0:D],
                        in0=ost_r[:, b0:b1, 0:D],
                        in1=bias_mat[:].unsqueeze(1).to_broadcast([P, nb, D]),
                        op=mybir.AluOpType.add)
                    nc.sync.dma_start(
                        out_d[:].rearrange("(b p) c -> p b c", p=P)[:, b0:b1],
                        ost_r[:, b0:b1, 0:D])

    nc.compile()
    return nc


def _wrap16(seg):
    """dma_gather idx layout: entry i at [i%16, i//16], replicated to the
    8 groups of 16 partitions."""
    n = len(seg)
    w = seg.reshape(n // 16, 16).T  # [16, n/16]
    return np.tile(w, (8, 1))


def _prep_host(in_feat, edge_ind, W_proj, a_src, a_tgt, bias):
    import ml_dtypes
    bfd = ml_dtypes.bfloat16
    f8 = ml_dtypes.float8_e4m3

    src = np.asarray(edge_ind[0]).astype(np.int64)
    tgt = np.asarray(edge_ind[1]).astype(np.int64)
    src_r = _r_of(src)
    tgt_r = _r_of(tgt)

    xT = np.zeros((P, NPAD), bfd)
    xT[:, :N_NODES] = np.asarray(in_feat, np.float32).T.astype(bfd)
    W = np.ascontiguousarray(np.asarray(W_proj, np.float32).astype(bfd))
    WT = np.ascontiguousarray(W.T)
    A = np.zeros((P, 2 * H), bfd)
    a_src = np.asarray(a_src, np.float32)
    a_tgt = np.asarray(a_tgt, np.float32)
    for h in range(H):
        A[h * 32:(h + 1) * 32, h] = a_src[0, h]
        A[h * 32:(h + 1) * 32, H + h] = a_tgt[0, h]
    # bias pre-interleaved to (r, h) column order
    b_np = np.asarray(bias, np.float32).reshape(H, D // H)
    bias_row = np.ascontiguousarray(b_np.T.reshape(1, D))

    # ---- assign each real target row to (core, block, slot) ----
    is_lo_t = tgt_r < SPLIT
    core = np.where(is_lo_t, tgt_r // LO_TPC, (tgt_r - SPLIT) // HI_TPC)
    src_is_lo = src_r < SPLIT
    deg_lo = np.bincount(tgt_r[src_is_lo], minlength=NPAD).astype(np.int64)
    deg_hi = np.bincount(tgt_r[~src_is_lo], minlength=NPAD).astype(np.int64)
    is_real = np.zeros(NPAD, bool)
    is_real[_r_of(np.arange(N_NODES))] = True

    blk_of = np.full(NPAD, -1, np.int32)
    tin_of = np.full(NPAD, -1, np.int32)
    for c in range(N_CORES):
        for base, n_t, b0, nb in (
                (c * LO_TPC, LO_TPC, 0, LO_NBLK),
                (SPLIT + c * HI_TPC, HI_TPC, LO_NBLK, HI_NBLK)):
            ids = np.arange(base, base + n_t)
            ids = ids[is_real[ids]]
            order = np.argsort(-(deg_lo[ids] + deg_hi[ids]), kind="stable")
            loads_l = np.zeros(nb, np.int64)
            loads_h = np.zeros(nb, np.int64)
            fill = np.zeros(nb, np.int32)
            for t in ids[order]:
                cand = np.nonzero(fill < P)[0]
                # normalize by per-side tile budgets so neither side's
                # global max tile count exceeds its cdiv-ideal
                j = cand[np.argmin(np.maximum(
                    (loads_l[cand] + deg_lo[t]) * (1.0 / 1408.0),
                    (loads_h[cand] + deg_hi[t]) * (1.0 / 768.0))
                    + 1e-5 * fill[cand])]
                blk_of[t] = b0 + j
                tin_of[t] = fill[j]
                fill[j] += 1
                loads_l[j] += deg_lo[t]
                loads_h[j] += deg_hi[t]
    blk = blk_of[tgt_r]
    tin = tin_of[tgt_r]

    key = (core * NBLK + blk).astype(np.int64)
    n_lo_e = np.bincount(key[src_is_lo], minlength=N_CORES * NBLK)
    n_hi_e = np.bincount(key[~src_is_lo], minlength=N_CORES * NBLK)
    k_lo = max(1, cdiv(int(n_lo_e.max()), P))
    k_hi = max(1, cdiv(int(n_hi_e.max()), P))
    T_B = k_lo + k_hi
    IW = T_B * 8
    SW = T_B * P

    core_inputs = []
    shared = {"xT": xT, "W": W, "WT": WT, "A": A, "bias": bias_row}
    out_perm = np.full((N_CORES, NBLK * P), -1, np.int64)
    rows_all = np.arange(NPAD)
    node_of_r = (rows_all % NT_NODE) * P + rows_all // NT_NODE
    for c in range(N_CORES):
        sel = np.nonzero((blk_of >= 0)
                         & (((rows_all < SPLIT) & (rows_all // LO_TPC == c))
                            | ((rows_all >= SPLIT)
                               & ((rows_all - SPLIT) // HI_TPC == c))))[0]
        out_perm[c, blk_of[sel] * P + tin_of[sel]] = node_of_r[sel]
        # per-block slot -> target row (for the tiny target-score gather)
        q16 = np.zeros((P, NBLK * 8), np.int16)
        qrow = np.zeros((NBLK, P), np.int64)
        qrow[blk_of[sel], tin_of[sel]] = np.where(sel < SPLIT, sel, sel - SPLIT)
        for b in range(NBLK):
            q16[:, b * 8:(b + 1) * 8] = _wrap16(qrow[b].astype(np.int16))

        m = core == c
        cs, cb, ct, clo = src_r[m], blk[m], tin[m], src_is_lo[m]
        s16 = np.zeros((P, NBLK * IW), np.int16)
        tinb = np.full((NBLK, T_B * P), -1, np.int16)
        for b in range(NBLK):
            mb_ = cb == b
            lo_sel = mb_ & clo
            hi_sel = mb_ & ~clo
            nl, nh = int(lo_sel.sum()), int(hi_sel.sum())
            sseg = np.zeros(T_B * P, np.int16)
            sseg[:nl] = cs[lo_sel].astype(np.int16)
            sseg[k_lo * P:k_lo * P + nh] = (cs[hi_sel] - SPLIT).astype(np.int16)
            tinb[b, :nl] = ct[lo_sel].astype(np.int16)
            tinb[b, k_lo * P:k_lo * P + nh] = ct[hi_sel].astype(np.int16)
            s16[:, b * IW:b * IW + k_lo * 8] = _wrap16(sseg[:k_lo * P])
            s16[:, b * IW + k_lo * 8:(b + 1) * IW] = _wrap16(sseg[k_lo * P:])
        # one-hots: S8[p, b, j, q] scatters edge (p of tile j) to target q;
        # St8[q, b, j, p] broadcasts target q's score to edge p of tile j
        S8 = np.zeros((P, NBLK, T_B, P), f8)
        St8 = np.zeros((P, NBLK, T_B, P), f8)
        bi, jp = np.nonzero(tinb >= 0)
        pp = jp % P
        jj = jp // P
        qq = tinb[bi, jp].astype(np.int64)
        S8[pp, bi, jj, qq] = 1.0
        St8[qq, bi, jj, pp] = 1.0
        core_inputs.append({**shared, "sidx": s16, "qidx": q16,
                            "S8": S8.reshape(P, NBLK * SW),
                            "St8": St8.reshape(P, NBLK * SW)})
    return k_lo, k_hi, core_inputs, out_perm


def kernel(in_feat, edge_ind, edge_len, W_proj, a_src, a_tgt, bias):
    k_lo, k_hi, core_inputs, out_perm = _prep_host(in_feat, edge_ind, W_proj,
                                                   a_src, a_tgt, bias)
    if (k_lo, k_hi) not in _cache:
        _cache[(k_lo, k_hi)] = _build(k_lo, k_hi)
    nc = _cache[(k_lo, k_hi)]

    from concourse.bass_utils import run_bass_kernel_spmd
    res = run_bass_kernel_spmd(nc, core_inputs, list(range(N_CORES)))

    out = np.zeros((N_NODES, D), np.float32)
    # undo head interleave: device col r*4+h = feature h*32+r
    unshuf = (np.arange(D).reshape(D // H, H).T.reshape(-1))
    for c in range(N_CORES):
        o = res.results[c]["out"]
        valid = out_perm[c] >= 0
        out[out_perm[c][valid]] = o[valid][:, unshuf]
    return out
